# revision 7
# baseline (speedup 1.0000x reference)
"""Conv4d (3,3,3,3) kernel for Trainium2, 8 NeuronCores.

Problem: x (2,24,16,16,48,48) * weight (48,24,3,3,3,3) + bias3d.sum(0)
      -> out (2,48,14,14,46,46), stride 1, no padding.

Strategy
--------
Sharding: 8 cores = (batch 2) x (ol-block 2) x (od-block 2). Each core owns a
7x7 block of (ol, od) output planes (49 tasks).

Per task: implicit GEMM. Contraction rows = (lo, do, ci) = 216 (+1 bias row),
packed on the host into xs[t, 217, 48, 48] where row r = (lo*3+do)*24+ci is
the input plane x[b, ci, ol+lo, od+do, :, :]; row 216 is all-ones. For each
of the 9 (ho, wo) kernel offsets the moving operand is the same SBUF-resident
tile sliced [k, oh0+ho : oh0+ho+rows, wo : wo+46]; all offsets accumulate
into one PSUM tile of output rows [48, rows, 46]. Bias is weight row 216
(offset (0,0) only) against the ones row.

dtype fp16 (default): 1 cycle/row on the PE, ~3e-4 scale-relative error
after fp32 PSUM accumulation over 1944 terms (weights/activations are well
inside fp16 range). CONV_DTYPE=f32r gives full fp32 operand storage at the
same matmul rate (~1.4e-4) at 2x the DMA bytes.

Measured on HW (repeat-loop delta, 8 cores): ~1.0 ms per kernel execution,
~90 matmuls x ~460-element streams per task, PE-serial bound. Column-half
tile_position concurrency and weight-stationary reorderings were measured
and gave no speedup in the full kernel (see CONV_COLSPLIT knob); input-DMA
SBUF writes account for ~16% of the span.
"""

import os
import sys

if "/opt/trn_rl_repo" not in sys.path:
    sys.path.insert(0, "/opt/trn_rl_repo")

from contextlib import nullcontext

import numpy as np

from concourse import bacc, bass, tile
from concourse.bass_utils import run_bass_kernel_spmd

mybir = bass.mybir

B, CI, CO = 2, 24, 48
L, D, H, W = 16, 16, 48, 48
OL, OD, OH, OW = 14, 14, 46, 46
N_TASKS = 49  # 7x7 (ol, od) planes per core
KROWS = 217  # (lo,do,ci) contraction rows + ones row
KSPLIT = 128  # k1 = rows 0:128, k2 = rows 128:217

# output row chunks: N = rows*46 <= 512 (PSUM bank), >= 256 (fp32r full rate)
if os.environ.get("CONV_CHUNKS"):
    CHUNK_ROWS = tuple(int(x) for x in os.environ["CONV_CHUNKS"].split(","))
    assert sum(CHUNK_ROWS) == 46
elif os.environ.get("CONV_CHUNK8", "0") == "1":
    CHUNK_ROWS = (8, 8, 8, 8, 8, 6)
else:
    CHUNK_ROWS = (10, 10, 10, 10, 6)
CHUNK_OH0 = tuple(sum(CHUNK_ROWS[:i]) for i in range(len(CHUNK_ROWS)))

ACTCOPY = os.environ.get("CONV_ACTCOPY", "0") == "1"
EPIALT = os.environ.get("CONV_EPIALT", "0") == "1"
_DTYPE_NAMES = {
    "f32r": mybir.dt.float32r,
    "bf16": mybir.dt.bfloat16,
    "f16": mybir.dt.float16,
    "f32": mybir.dt.float32,
    "f8e4": mybir.dt.float8e4,
}
DTYPE = _DTYPE_NAMES[os.environ.get("CONV_DTYPE", "f16")]
COLSPLIT = os.environ.get("CONV_COLSPLIT", "0") == "1"
# PITCH=46: exact-width 3D rhs/psum APs; PITCH=48: padded-width contiguous 1D
# rhs (junk output columns ow=46,47 dropped on host)
PITCH = int(os.environ.get("CONV_PITCH", "46"))
X_BUFS = int(os.environ.get("CONV_XBUFS", "3"))
PS_BUFS = int(os.environ.get("CONV_PSBUFS", "8"))
O_BUFS = int(os.environ.get("CONV_OBUFS", "6"))
# bench-only bisection knobs (break correctness!)
NOEPI = os.environ.get("CONV_NOEPI", "0") == "1"  # skip copy + dma-out
NOLOAD = os.environ.get("CONV_NOLOAD", "0") == "1"  # skip k-tile loads
HALFLOAD = os.environ.get("CONV_HALFLOAD", "0") == "1"  # k1 DMA, k2 memset
# which engine queue issues the input loads: sp (default), act, gpsimd
LOADQ = os.environ.get("CONV_LOADQ", "sp")
# defer epilogue emission one pair back (denser PE stream at boundaries)
LATEEPI = os.environ.get("CONV_LATEEPI", "0") == "1"


def _np_dtype():
    return mybir.dt.np(DTYPE)


def build_program(n_tasks: int = N_TASKS, repeat: int = 1):
    nc = bacc.Bacc()
    f32 = mybir.dt.float32
    k2rows = KROWS - KSPLIT  # 89
    p48 = PITCH == 48
    FPAD = 2306  # flat plane + 2 pad elems for the largest shifted read

    if p48:
        xs_d = nc.dram_tensor("xs", [n_tasks, KROWS, FPAD], DTYPE, kind="ExternalInput")
        out_d = nc.dram_tensor("out", [n_tasks, CO, OH, 48], f32, kind="ExternalOutput")
    else:
        xs_d = nc.dram_tensor("xs", [n_tasks, KROWS, H, W], DTYPE, kind="ExternalInput")
        out_d = nc.dram_tensor("out", [n_tasks, CO, OH, OW], f32, kind="ExternalOutput")
    w1_d = nc.dram_tensor("w1", [KSPLIT, 9, CO], DTYPE, kind="ExternalInput")
    w2_d = nc.dram_tensor("w2", [k2rows, 9, CO], DTYPE, kind="ExternalInput")

    with tile.TileContext(nc) as tc:
        with (
            tc.tile_pool(name="wpool", bufs=1) as wpool,
            tc.tile_pool(name="xpool", bufs=X_BUFS) as xpool,
            tc.tile_pool(name="opool", bufs=O_BUFS) as opool,
            tc.tile_pool(name="pspool", bufs=PS_BUFS, space="PSUM") as pspool,
            tc.For_i(0, repeat, 1) if repeat > 1 else nullcontext(),
        ):
            w1s = wpool.tile([KSPLIT, 9, CO], DTYPE)
            w2s = wpool.tile([k2rows, 9, CO], DTYPE)
            nc.sync.dma_start(out=w1s[:], in_=w1_d[:])
            nc.sync.dma_start(out=w2s[:], in_=w2_d[:])

            nchunk = len(CHUNK_ROWS)
            pending = []
            for t in range(n_tasks):
                kshape = [KSPLIT, FPAD] if p48 else [KSPLIT, H, W]
                k2shape = [k2rows, FPAD] if p48 else [k2rows, H, W]
                k1 = xpool.tile(kshape, DTYPE, tag="k1")
                k2 = xpool.tile(k2shape, DTYPE, tag="k2")
                ldq = {"sp": nc.sync, "act": nc.scalar, "gpsimd": nc.gpsimd}[LOADQ]
                if NOLOAD:
                    nc.vector.memset(k1[:], 0.25)
                    nc.vector.memset(k2[:], 0.25)
                elif HALFLOAD:
                    ldq.dma_start(out=k1[:], in_=xs_d[t, 0:KSPLIT])
                    nc.vector.memset(k2[:], 0.25)
                else:
                    ldq.dma_start(out=k1[:], in_=xs_d[t, 0:KSPLIT])
                    ldq.dma_start(out=k2[:], in_=xs_d[t, KSPLIT:KROWS])

                for ci0 in range(0, nchunk, 2):
                    pair = [ci0] + ([ci0 + 1] if ci0 + 1 < nchunk else [])
                    tcur = t
                    ps_l, o_l = [], []
                    for j, c in enumerate(pair):
                        rows = CHUNK_ROWS[c]
                        # psum tile is one full bank (512 f32) per partition so
                        # the partition-64 slice stays bank-aligned; matmul
                        # writes it as a flat [48, rows*46] AP
                        pw = 48 if p48 else OW
                        ps_full = pspool.tile([112, 512], f32, tag="ps")
                        o_full = opool.tile([112, 10, 48 if p48 else OW], f32, tag="o")
                        p0 = 64 if (COLSPLIT and j == 1) else 0
                        ps_l.append(ps_full[p0 : p0 + CO, 0 : rows * pw])
                        o_l.append(o_full[p0 : p0 + CO, :rows, :])

                    if True:
                        for idx in range(9):
                            ho, wo = divmod(idx, 3)
                            for kt, (ks, ws) in enumerate(((k1, w1s), (k2, w2s))):
                                for j, c in enumerate(pair):
                                    rows = CHUNK_ROWS[c]
                                    oh0 = CHUNK_OH0[c]
                                    if p48:
                                        off = (oh0 + ho) * 48 + wo
                                        rhs = ks[:, off : off + rows * 48]
                                    else:
                                        rhs = ks[:, oh0 + ho : oh0 + ho + rows, wo : wo + OW]
                                    nc.tensor.matmul(
                                        ps_l[j],
                                        lhsT=ws[:, idx, :],
                                        rhs=rhs,
                                        start=(idx == 0 and kt == 0),
                                        stop=(idx == 8 and kt == 1),
                                    )
                    def _epi(tt=tcur, pr=tuple(pair), psl=tuple(ps_l), ol=tuple(o_l)):
                        for j, c in enumerate(pr):
                            if NOEPI and not (tt == n_tasks - 1 and c == nchunk - 1):
                                continue
                            rows = CHUNK_ROWS[c]
                            if ACTCOPY or (EPIALT and c % 2 == 1):
                                nc.scalar.copy(out=ol[j], in_=psl[j])
                            else:
                                nc.vector.tensor_copy(out=ol[j], in_=psl[j])
                            nc.gpsimd.dma_start(
                                out=out_d[tt, :, CHUNK_OH0[c] : CHUNK_OH0[c] + rows, :],
                                in_=ol[j],
                            )

                    if LATEEPI:
                        pending.append(_epi)
                        if len(pending) > 1:
                            pending.pop(0)()
                    else:
                        _epi()
            for f in pending:
                f()
    nc.finalize()
    return nc


def make_in_maps(x, weight, bias3d, n_tasks: int = N_TASKS):
    """Host-side shard + repack into the per-task packed-row layout."""
    npdt = _np_dtype()
    x = np.asarray(x, np.float32)
    weight = np.asarray(weight, np.float32)
    bias3d = np.asarray(bias3d, np.float32)

    # W[(lo*3+do)*24+ci, ho*3+wo, co] = weight[co, ci, lo, do, ho, wo]
    Wr = np.ascontiguousarray(np.transpose(weight, (2, 3, 1, 4, 5, 0))).reshape(
        216, 9, CO
    )
    Wfull = np.zeros((KROWS, 9, CO), np.float32)
    Wfull[:216] = Wr
    Wfull[216, 0, :] = bias3d.sum(axis=0)
    w1 = np.ascontiguousarray(Wfull[:KSPLIT]).astype(npdt)
    w2 = np.ascontiguousarray(Wfull[KSPLIT:]).astype(npdt)

    in_maps = []
    for c in range(8):
        b, lb, db = c // 4, (c // 2) % 2, c % 2
        slab = np.ascontiguousarray(
            x[b, :, 7 * lb : 7 * lb + 9, 7 * db : 7 * db + 9]
        )  # (24, 9, 9, 48, 48)
        s_ci, s_l, s_d, s_h, s_w = slab.strides
        # V[l0, d0, lo, do, ci, h, w] = slab[ci, l0+lo, d0+do, h, w]
        V = np.lib.stride_tricks.as_strided(
            slab,
            shape=(7, 7, 3, 3, CI, H, W),
            strides=(s_l, s_d, s_l, s_d, s_ci, s_h, s_w),
        )
        if PITCH == 48:
            xs = np.zeros((N_TASKS, KROWS, 2306), np.float32)
            xs[:, :216, :2304] = V.reshape(N_TASKS, 216, H * W)
            xs[:, 216] = 1.0
        else:
            xs = np.empty((N_TASKS, KROWS, H, W), np.float32)
            xs[:, :216] = V.reshape(N_TASKS, 216, H, W)
            xs[:, 216] = 1.0
        in_maps.append({"xs": xs[:n_tasks].astype(npdt), "w1": w1, "w2": w2})
    return in_maps


def assemble_output(results):
    out = np.empty((B, CO, OL, OD, OH, OW), np.float32)
    for c in range(8):
        b, lb, db = c // 4, (c // 2) % 2, c % 2
        r = np.asarray(results[c]["out"]).reshape(7, 7, CO, OH, -1)[..., :OW]
        out[b, :, 7 * lb : 7 * lb + 7, 7 * db : 7 * db + 7] = r.transpose(2, 0, 1, 3, 4)
    return out


_NC_CACHE = {}


def _get_program():
    if "nc" not in _NC_CACHE:
        _NC_CACHE["nc"] = build_program()
    return _NC_CACHE["nc"]


def kernel(x, weight, bias3d):
    nc = _get_program()
    in_maps = make_in_maps(x, weight, bias3d)
    res = run_bass_kernel_spmd(nc, in_maps, list(range(8))).results
    return assemble_output(res)



# revision 23
# speedup vs baseline: 1.2835x; 1.2835x over previous
"""Conv4d (3,3,3,3) kernel for Trainium2, 8 NeuronCores.

Problem: x (2,24,16,16,48,48) * weight (48,24,3,3,3,3) + bias3d.sum(0)
      -> out (2,48,14,14,46,46), stride 1, no padding.

Strategy
--------
Sharding: 8 cores = (batch 2) x (ol-block 2) x (od-block 2). Each core owns a
7x7 block of (ol, od) output planes (49 tasks).

Per task: implicit GEMM. Contraction rows = (lo, do, ci) = 216 (+1 bias row),
packed on the host into xs[t, 217, 48, 48] where row r = (lo*3+do)*24+ci is
the input plane x[b, ci, ol+lo, od+do, :, :]; row 216 is all-ones. For each
of the 9 (ho, wo) kernel offsets the moving operand is the same SBUF-resident
tile sliced [k, oh0+ho : oh0+ho+rows, wo : wo+46]; all offsets accumulate
into one PSUM tile of output rows [48, rows, 46]. Bias is weight row 216
(offset (0,0) only) against the ones row.

dtype fp16 (default): 1 cycle/row on the PE, ~3e-4 scale-relative error
after fp32 PSUM accumulation over 1944 terms (weights/activations are well
inside fp16 range). CONV_DTYPE=f32r gives full fp32 operand storage at the
same matmul rate (~1.4e-4) at 2x the DMA bytes.

Measured on HW (repeat-loop delta, 8 cores): ~1.0 ms per kernel execution,
~90 matmuls x ~460-element streams per task, PE-serial bound. Column-half
tile_position concurrency and weight-stationary reorderings were measured
and gave no speedup in the full kernel (see CONV_COLSPLIT knob); input-DMA
SBUF writes account for ~16% of the span.
"""

import os
import sys

if "/opt/trn_rl_repo" not in sys.path:
    sys.path.insert(0, "/opt/trn_rl_repo")

from contextlib import nullcontext

import numpy as np

from concourse import bacc, bass, tile
from concourse.bass_utils import run_bass_kernel_spmd

mybir = bass.mybir

B, CI, CO = 2, 24, 48
L, D, H, W = 16, 16, 48, 48
OL, OD, OH, OW = 14, 14, 46, 46
N_TASKS = 49  # 7x7 (ol, od) planes per core
KROWS = 217  # (lo,do,ci) contraction rows + ones row
KSPLIT = 128  # k1 = rows 0:128, k2 = rows 128:217

# output row chunks: N = rows*46 <= 512 (PSUM bank), >= 256 (fp32r full rate)
if os.environ.get("CONV_CHUNKS"):
    CHUNK_ROWS = tuple(int(x) for x in os.environ["CONV_CHUNKS"].split(","))
    assert sum(CHUNK_ROWS) == 46
elif os.environ.get("CONV_CHUNK8", "0") == "1":
    CHUNK_ROWS = (8, 8, 8, 8, 8, 6)
else:
    CHUNK_ROWS = (10, 10, 10, 10, 6)
CHUNK_OH0 = tuple(sum(CHUNK_ROWS[:i]) for i in range(len(CHUNK_ROWS)))

ACTCOPY = os.environ.get("CONV_ACTCOPY", "0") == "1"
EPIALT = os.environ.get("CONV_EPIALT", "0") == "1"
_DTYPE_NAMES = {
    "f32r": mybir.dt.float32r,
    "bf16": mybir.dt.bfloat16,
    "f16": mybir.dt.float16,
    "f32": mybir.dt.float32,
    "f8e4": mybir.dt.float8e4,
}
DTYPE = _DTYPE_NAMES[os.environ.get("CONV_DTYPE", "f16")]
COLSPLIT = os.environ.get("CONV_COLSPLIT", "0") == "1"
# PITCH=46: exact-width 3D rhs/psum APs; PITCH=48: padded-width contiguous 1D
# rhs (junk output columns ow=46,47 dropped on host)
PITCH = int(os.environ.get("CONV_PITCH", "46"))
X_BUFS = int(os.environ.get("CONV_XBUFS", "3"))
PS_BUFS = int(os.environ.get("CONV_PSBUFS", "8"))
O_BUFS = int(os.environ.get("CONV_OBUFS", "6"))
# bench-only bisection knobs (break correctness!)
NOEPI = os.environ.get("CONV_NOEPI", "0") == "1"  # skip copy + dma-out
NOLOAD = os.environ.get("CONV_NOLOAD", "0") == "1"  # skip k-tile loads
HALFLOAD = os.environ.get("CONV_HALFLOAD", "0") == "1"  # k1 DMA, k2 memset
# which engine queue issues the input loads: sp (default), act, gpsimd
LOADQ = os.environ.get("CONV_LOADQ", "sp")
# defer epilogue emission one pair back (denser PE stream at boundaries)
LATEEPI = os.environ.get("CONV_LATEEPI", "0") == "1"


def _np_dtype():
    return mybir.dt.np(DTYPE)


def build_program(n_tasks: int = N_TASKS, repeat: int = 1):
    nc = bacc.Bacc()
    f32 = mybir.dt.float32
    k2rows = KROWS - KSPLIT  # 89
    p48 = PITCH == 48
    FPAD = 2306  # flat plane + 2 pad elems for the largest shifted read

    if p48:
        xs_d = nc.dram_tensor("xs", [n_tasks, KROWS, FPAD], DTYPE, kind="ExternalInput")
        out_d = nc.dram_tensor("out", [n_tasks, CO, OH, 48], f32, kind="ExternalOutput")
    else:
        xs_d = nc.dram_tensor("xs", [n_tasks, KROWS, H, W], DTYPE, kind="ExternalInput")
        out_d = nc.dram_tensor("out", [n_tasks, CO, OH, OW], f32, kind="ExternalOutput")
    w1_d = nc.dram_tensor("w1", [KSPLIT, 9, CO], DTYPE, kind="ExternalInput")
    w2_d = nc.dram_tensor("w2", [k2rows, 9, CO], DTYPE, kind="ExternalInput")

    with tile.TileContext(nc) as tc:
        with (
            tc.tile_pool(name="wpool", bufs=1) as wpool,
            tc.tile_pool(name="xpool", bufs=X_BUFS) as xpool,
            tc.tile_pool(name="opool", bufs=O_BUFS) as opool,
            tc.tile_pool(name="pspool", bufs=PS_BUFS, space="PSUM") as pspool,
            tc.For_i(0, repeat, 1) if repeat > 1 else nullcontext(),
        ):
            w1s = wpool.tile([KSPLIT, 9, CO], DTYPE)
            w2s = wpool.tile([k2rows, 9, CO], DTYPE)
            nc.sync.dma_start(out=w1s[:], in_=w1_d[:])
            nc.sync.dma_start(out=w2s[:], in_=w2_d[:])

            nchunk = len(CHUNK_ROWS)
            pending = []
            for t in range(n_tasks):
                kshape = [KSPLIT, FPAD] if p48 else [KSPLIT, H, W]
                k2shape = [k2rows, FPAD] if p48 else [k2rows, H, W]
                k1 = xpool.tile(kshape, DTYPE, tag="k1")
                k2 = xpool.tile(k2shape, DTYPE, tag="k2")
                ldq = {"sp": nc.sync, "act": nc.scalar, "gpsimd": nc.gpsimd}[LOADQ]
                if NOLOAD:
                    nc.vector.memset(k1[:], 0.25)
                    nc.vector.memset(k2[:], 0.25)
                elif HALFLOAD:
                    ldq.dma_start(out=k1[:], in_=xs_d[t, 0:KSPLIT])
                    nc.vector.memset(k2[:], 0.25)
                else:
                    ldq.dma_start(out=k1[:], in_=xs_d[t, 0:KSPLIT])
                    ldq.dma_start(out=k2[:], in_=xs_d[t, KSPLIT:KROWS])

                for ci0 in range(0, nchunk, 2):
                    pair = [ci0] + ([ci0 + 1] if ci0 + 1 < nchunk else [])
                    tcur = t
                    ps_l, o_l = [], []
                    for j, c in enumerate(pair):
                        rows = CHUNK_ROWS[c]
                        # psum tile is one full bank (512 f32) per partition so
                        # the partition-64 slice stays bank-aligned; matmul
                        # writes it as a flat [48, rows*46] AP
                        pw = 48 if p48 else OW
                        ps_full = pspool.tile([112, 512], f32, tag="ps")
                        o_full = opool.tile([112, 10, 48 if p48 else OW], f32, tag="o")
                        p0 = 64 if (COLSPLIT and j == 1) else 0
                        ps_l.append(ps_full[p0 : p0 + CO, 0 : rows * pw])
                        o_l.append(o_full[p0 : p0 + CO, :rows, :])

                    if True:
                        for idx in range(9):
                            ho, wo = divmod(idx, 3)
                            for kt, (ks, ws) in enumerate(((k1, w1s), (k2, w2s))):
                                for j, c in enumerate(pair):
                                    rows = CHUNK_ROWS[c]
                                    oh0 = CHUNK_OH0[c]
                                    if p48:
                                        off = (oh0 + ho) * 48 + wo
                                        rhs = ks[:, off : off + rows * 48]
                                    else:
                                        rhs = ks[:, oh0 + ho : oh0 + ho + rows, wo : wo + OW]
                                    nc.tensor.matmul(
                                        ps_l[j],
                                        lhsT=ws[:, idx, :],
                                        rhs=rhs,
                                        start=(idx == 0 and kt == 0),
                                        stop=(idx == 8 and kt == 1),
                                    )
                    def _epi(tt=tcur, pr=tuple(pair), psl=tuple(ps_l), ol=tuple(o_l)):
                        for j, c in enumerate(pr):
                            if NOEPI and not (tt == n_tasks - 1 and c == nchunk - 1):
                                continue
                            rows = CHUNK_ROWS[c]
                            if ACTCOPY or (EPIALT and c % 2 == 1):
                                nc.scalar.copy(out=ol[j], in_=psl[j])
                            else:
                                nc.vector.tensor_copy(out=ol[j], in_=psl[j])
                            nc.gpsimd.dma_start(
                                out=out_d[tt, :, CHUNK_OH0[c] : CHUNK_OH0[c] + rows, :],
                                in_=ol[j],
                            )

                    if LATEEPI:
                        pending.append(_epi)
                        if len(pending) > 1:
                            pending.pop(0)()
                    else:
                        _epi()
            for f in pending:
                f()
    nc.finalize()
    return nc


def make_in_maps(x, weight, bias3d, n_tasks: int = N_TASKS):
    """Host-side shard + repack into the per-task packed-row layout."""
    npdt = _np_dtype()
    x = np.asarray(x, np.float32)
    weight = np.asarray(weight, np.float32)
    bias3d = np.asarray(bias3d, np.float32)

    # W[(lo*3+do)*24+ci, ho*3+wo, co] = weight[co, ci, lo, do, ho, wo]
    Wr = np.ascontiguousarray(np.transpose(weight, (2, 3, 1, 4, 5, 0))).reshape(
        216, 9, CO
    )
    Wfull = np.zeros((KROWS, 9, CO), np.float32)
    Wfull[:216] = Wr
    Wfull[216, 0, :] = bias3d.sum(axis=0)
    w1 = np.ascontiguousarray(Wfull[:KSPLIT]).astype(npdt)
    w2 = np.ascontiguousarray(Wfull[KSPLIT:]).astype(npdt)

    in_maps = []
    for c in range(8):
        b, lb, db = c // 4, (c // 2) % 2, c % 2
        slab = np.ascontiguousarray(
            x[b, :, 7 * lb : 7 * lb + 9, 7 * db : 7 * db + 9]
        )  # (24, 9, 9, 48, 48)
        s_ci, s_l, s_d, s_h, s_w = slab.strides
        # V[l0, d0, lo, do, ci, h, w] = slab[ci, l0+lo, d0+do, h, w]
        V = np.lib.stride_tricks.as_strided(
            slab,
            shape=(7, 7, 3, 3, CI, H, W),
            strides=(s_l, s_d, s_l, s_d, s_ci, s_h, s_w),
        )
        if PITCH == 48:
            xs = np.zeros((N_TASKS, KROWS, 2306), np.float32)
            xs[:, :216, :2304] = V.reshape(N_TASKS, 216, H * W)
            xs[:, 216] = 1.0
        else:
            xs = np.empty((N_TASKS, KROWS, H, W), np.float32)
            xs[:, :216] = V.reshape(N_TASKS, 216, H, W)
            xs[:, 216] = 1.0
        in_maps.append({"xs": xs[:n_tasks].astype(npdt), "w1": w1, "w2": w2})
    return in_maps


def assemble_output(results):
    out = np.empty((B, CO, OL, OD, OH, OW), np.float32)
    for c in range(8):
        b, lb, db = c // 4, (c // 2) % 2, c % 2
        r = np.asarray(results[c]["out"]).reshape(7, 7, CO, OH, -1)[..., :OW]
        out[b, :, 7 * lb : 7 * lb + 7, 7 * db : 7 * db + 7] = r.transpose(2, 0, 1, 3, 4)
    return out


# ---------------------------------------------------------------------------
# v2: d-major od-grouped packing.
#
# Each core owns a 7x7 (ol, od) grid of output planes. Tasks sharing an ol row
# reuse most input planes, so v2 ships each ol-row's planes once, grouped along
# od into a 3-window group (od 0-2, d-planes 0-4, 360 rows) and a 4-window
# group (od 3-6, d-planes 3-8, 432 rows). Rows are d-major: r = do'*72 +
# lo*24 + ci. Window j of a group contracts rows [72j, 72j+216) -- contiguous,
# so each window's matmuls are 2-3 full-tile (k=128) pieces whose weight slots
# are zero-padded outside the window. This cuts DMA-in ~48% (49 -> 25.6 MB per
# core), which keeps the 2-way PE column-tiling concurrency alive (measured:
# >~30MB/exec of DMA-in collapses the tile overlap and serializes the PE).
#
# Chunks (8,8,8,8,7,7) of output rows pair onto PE column halves (0,0)/(0,64)
# for the 2x stream concurrency; psum pairs share one [112,512] bank tile.
# Bias is added on the host; output DMAs as f16.
# ---------------------------------------------------------------------------

V2_CHUNK_ROWS = (8, 8, 8, 8, 7, 7)
V2_CHUNK_OH0 = (0, 8, 16, 24, 32, 39)
V2_PAIRS = ((0, 1), (2, 3), (4, 5))
V2_NOEPI = os.environ.get("CONV_V2_NOEPI", "0") == "1"  # bench-only
V2_NOLOAD = os.environ.get("CONV_V2_NOLOAD", "0") == "1"  # bench-only
V2_LATEEPI = os.environ.get("CONV_V2_LATEEPI", "0") == "1"
GROUPS = (  # (od0, n_windows, n_dplanes)
    (0, 3, 5),
    (3, 4, 6),
)


def _v2_pieces(n_windows):
    """Per window j: list of (tile_idx, klen) pieces covering rows
    [72j, 72j+216) with full-tile APs (weights zero-padded)."""
    nrows = 72 * (n_windows + 2)
    ntiles = (nrows + 127) // 128
    out = []
    for j in range(n_windows):
        lo, hi = 72 * j, 72 * j + 216
        pieces = []
        for k in range(ntiles):
            t0, t1 = 128 * k, min(128 * k + 128, nrows)
            if t0 < hi and t1 > lo:  # overlap
                klen = t1 - t0
                if n_windows == 4 and k == 3:
                    klen = 128  # memset-padded tail tile
                elif klen < 72:
                    klen = min(128, nrows - t0)
                pieces.append((k, klen))
        out.append(pieces)
    return out


def build_program_v2(n_olrows: int = 7, repeat: int = 1):
    nc = bacc.Bacc()
    f32 = mybir.dt.float32
    f16 = mybir.dt.float16

    pcs = {3: _v2_pieces(3), 4: _v2_pieces(4)}
    nslots = sum(len(p) for n in (3, 4) for p in pcs[n])  # 7 + 10

    xsA_d = nc.dram_tensor("xsA", [n_olrows, 360, H, W], f16, kind="ExternalInput")
    xsB_d = nc.dram_tensor("xsB", [n_olrows, 432, H, W], f16, kind="ExternalInput")
    w_d = nc.dram_tensor("wall", [128, nslots, 9, CO], f16, kind="ExternalInput")
    out_d = nc.dram_tensor("out", [n_olrows, 7, CO, OH, OW], f16, kind="ExternalOutput")

    with tile.TileContext(nc) as tc:
        with (
            tc.tile_pool(name="wpool", bufs=1) as wpool,
            tc.tile_pool(name="xpool", bufs=int(os.environ.get("CONV_V2_XBUFS", "3"))) as xpool,
            tc.tile_pool(name="opool", bufs=2) as opool,
            tc.tile_pool(name="pspool", bufs=1, space="PSUM") as pspool,
            tc.For_i(0, repeat, 1) if repeat > 1 else nullcontext(),
        ):
            wall = wpool.tile([128, nslots, 9, CO], f16)
            nc.sync.dma_start(out=wall[:], in_=w_d[:])

            pending = []
            for ol in range(n_olrows):
                slot_base = 0
                for gi, (od0, nwin, ndp) in enumerate(GROUPS):
                    nrows = 72 * (nwin + 2)
                    ntiles = (nrows + 127) // 128
                    xs_d = xsA_d if gi == 0 else xsB_d
                    xt = []
                    for k in range(ntiles):
                        t = xpool.tile([128, H, W], f16, tag=f"x{gi}{k}")
                        t0 = 128 * k
                        t1 = min(t0 + 128, nrows)
                        if gi == 1 and k == 3:
                            # zero-fill the 80 tail rows (read by k=128 piece
                            # APs against zero weights); memset full tile,
                            # then DMA overwrites the real rows. gpsimd keeps
                            # DVE free for epilogue copies.
                            nc.gpsimd.memset(t[:], 0.0)
                        if V2_NOLOAD:
                            if not (gi == 1 and k == 3):
                                nc.gpsimd.memset(t[:], 0.25)
                        else:
                            nc.sync.dma_start(out=t[0 : t1 - t0], in_=xs_d[ol, t0:t1])
                        xt.append(t)

                    for j in range(nwin):
                        pieces = pcs[nwin][j]
                        slot_of = {}
                        sb = slot_base
                        for jj in range(nwin):
                            for q, (k, klen) in enumerate(pcs[nwin][jj]):
                                if jj == j:
                                    slot_of[k] = sb
                                sb += 1

                        ps_l, o_l = [], []
                        for pi, (cA, cB) in enumerate(V2_PAIRS):
                            # separate psum banks per column half: two
                            # accumulation groups may not share a bank
                            ps = {}
                            for half in (0, 64):
                                pst = pspool.tile(
                                    [112, 512], f32, tag=f"ps{pi}h{half}", name=f"ps{pi}h{half}"
                                )
                                ps[half] = pst
                            o = opool.tile([112, 8, OW], f16, tag=f"o{pi}")
                            ps_l.append(ps)
                            o_l.append(o)

                        for idx in range(9):
                            ho, wo = divmod(idx, 3)
                            for pi, (cA, cB) in enumerate(V2_PAIRS):
                                for np_, (k, klen) in enumerate(pieces):
                                    first = idx == 0 and np_ == 0
                                    last = idx == 8 and np_ == len(pieces) - 1
                                    lhsT = wall[0:klen, slot_of[k], idx, :]
                                    for half, c in ((0, cA), (64, cB)):
                                        rows = V2_CHUNK_ROWS[c]
                                        oh0 = V2_CHUNK_OH0[c]
                                        rhs = xt[k][
                                            0:klen,
                                            oh0 + ho : oh0 + ho + rows,
                                            wo : wo + OW,
                                        ]
                                        nc.tensor.matmul(
                                            ps_l[pi][half][
                                                half : half + CO, 0 : rows * OW
                                            ],
                                            lhsT=lhsT,
                                            rhs=rhs,
                                            start=first,
                                            stop=last,
                                            tile_position=(0, half),
                                        )

                        def _epi(
                            ol=ol, od=od0 + j, psl=tuple(ps_l), ol_=tuple(o_l), last=False
                        ):
                            if V2_NOEPI and not last:
                                return
                            for pi, (cA, cB) in enumerate(V2_PAIRS):
                                for half, c in ((0, cA), (64, cB)):
                                    rows = V2_CHUNK_ROWS[c]
                                    src = psl[pi][half][half : half + CO, 0 : rows * OW]
                                    dst = ol_[pi][half : half + CO, :rows, :]
                                    if (pi + (half > 0)) % 2 == 0:
                                        nc.vector.tensor_copy(out=dst, in_=src)
                                    else:
                                        nc.scalar.copy(out=dst, in_=src)
                                    # HWDGE (ACT ring): SWDGE out-DMAs can be
                                    # descriptor-starved while DVE holds the
                                    # shared SBUF port for psum evacuation
                                    nc.scalar.dma_start(
                                        out=out_d[
                                            ol,
                                            od,
                                            :,
                                            V2_CHUNK_OH0[c] : V2_CHUNK_OH0[c] + rows,
                                            :,
                                        ],
                                        in_=dst,
                                    )

                        if V2_LATEEPI:
                            pending.append(_epi)
                            if len(pending) > 1:
                                pending.pop(0)()
                        else:
                            _epi()
                    slot_base = sb
            for i, f in enumerate(pending):
                f(last=i == len(pending) - 1)
    nc.finalize()
    return nc


def make_in_maps_v2(x, weight, bias3d, n_olrows: int = 7):
    x = np.asarray(x, np.float32)
    weight = np.asarray(weight, np.float32)

    # W216[(do,lo,ci), idx=(ho*3+wo), co] = weight[co, ci, lo, do, ho, wo]
    W216 = np.ascontiguousarray(np.transpose(weight, (3, 2, 1, 4, 5, 0))).reshape(
        216, 9, CO
    )

    pcs = {3: _v2_pieces(3), 4: _v2_pieces(4)}
    slots = []
    for nwin in (3, 4):
        for j in range(nwin):
            for k, klen in pcs[nwin][j]:
                slots.append((nwin, j, k, klen))
    nslots = len(slots)
    wall = np.zeros((128, nslots, 9, CO), np.float32)
    for q, (nwin, j, k, klen) in enumerate(slots):
        for r in range(klen):
            g = 128 * k + r - 72 * j
            if 0 <= g < 216:
                wall[r, q] = W216[g]
    wall = wall.astype(np.float16)

    in_maps = []
    for c in range(8):
        b, lb, db = c // 4, (c // 2) % 2, c % 2
        slab = np.ascontiguousarray(
            x[b, :, 7 * lb : 7 * lb + 9, 7 * db : 7 * db + 9]
        )  # (24ci, 9l, 9d, 48, 48)
        xsA = np.empty((n_olrows, 360, H, W), np.float32)
        xsB = np.empty((n_olrows, 432, H, W), np.float32)
        for ol in range(n_olrows):
            for (od0, nwin, ndp), dst in zip(GROUPS, (xsA, xsB)):
                # rows r = do'*72 + lo*24 + ci -> slab[ci, ol+lo, od0+do']
                v = slab[:, ol : ol + 3, od0 : od0 + ndp]  # (24, 3, ndp, H, W)
                dst[ol] = np.transpose(v, (2, 1, 0, 3, 4)).reshape(72 * ndp, H, W)
        in_maps.append(
            {
                "xsA": xsA.astype(np.float16),
                "xsB": xsB.astype(np.float16),
                "wall": wall,
            }
        )
    return in_maps


def assemble_output_v2(results, bias3d):
    total_bias = np.asarray(bias3d, np.float32).sum(axis=0)  # (CO,)
    out = np.empty((B, CO, OL, OD, OH, OW), np.float32)
    for c in range(8):
        b, lb, db = c // 4, (c // 2) % 2, c % 2
        r = np.asarray(results[c]["out"], np.float32)  # (7, 7, CO, OH, OW)
        out[b, :, 7 * lb : 7 * lb + 7, 7 * db : 7 * db + 7] = r.transpose(2, 0, 1, 3, 4)
    out += total_bias[None, :, None, None, None, None]
    return out


V2 = os.environ.get("CONV_V2", "0") == "1"

_NC_CACHE = {}


def _get_program():
    if "nc" not in _NC_CACHE:
        _NC_CACHE["nc"] = build_program_v2() if V2 else build_program()
    return _NC_CACHE["nc"]


def kernel(x, weight, bias3d):
    nc = _get_program()
    if V2:
        in_maps = make_in_maps_v2(x, weight, bias3d)
        res = run_bass_kernel_spmd(nc, in_maps, list(range(8))).results
        return assemble_output_v2(res, bias3d)
    in_maps = make_in_maps(x, weight, bias3d)
    res = run_bass_kernel_spmd(nc, in_maps, list(range(8))).results
    return assemble_output(res)

